# revision 37
# baseline (speedup 1.0000x reference)
"""Distributed causal-attention kernel for 8 TRN2 NeuronCores.

Problem: B=4, S=1024, D=1024, H=16 causal attention block returning
(a, w) where a = proj(attn output) and w = softmax attention probs.

Sharding (tensor-parallel heads x data-parallel batch):
  core c -> (batch b = c//2, head-group g = c%2) ; each group = 8 heads.
  Each core computes its group's QKV projection, causal softmax
  (writing its [8, S, S] slice of w), A@V, and a partial output
  projection a_part = attn_g @ w_proj[rows g].  Host sums the two
  partials per batch (a = part0 + part1 + b_proj + b_attn_v @ w_proj).
  No device collectives needed.

Compute is bf16 on the TensorEngine (f32 PSUM accumulation); softmax
exp runs in f32 on the ScalarEngine.  The 1/sqrt(HD)=0.125 score scale
is folded into the Q weights on the host (exact, power of two).
Scores are computed in both [q,k] (for the softmax/w output) and
[k,q] (for the A@V matmul) orientations -- cheaper on TRN2 than any
transpose path.  Causal masking zeroes the diagonal-block upper
triangle on GpSimd after the exp; softmax row-sums ride along as a
ones-column in the A@V matmul, so the masked upper region is never
computed.  Upper-triangular w stays zero: output DRAM buffers are
zero-initialized by the runtime, so that region is never written.
"""

import numpy as np
import ml_dtypes

B, S, D, H = 4, 1024, 1024, 16
HD = D // H          # 64 head dim
G = H // 2           # 8 heads per core
GD = G * HD          # 512 dims per head group
VW = HD + 1          # v columns per head incl. ones column
NB = S // 128        # 8 token blocks
DK = D // 128        # 8 contraction tiles
FT = GD // 128       # 4 feature tiles per group
NCORES = 8
NEG = 10000.0

_CACHE = {}


def _build_bass():
    from contextlib import ExitStack
    import concourse.bass as bass
    import concourse.tile as tile
    from concourse import bacc, mybir
    from concourse.masks import make_identity

    f32 = mybir.dt.float32
    bf16 = mybir.dt.bfloat16
    ts = bass.ts
    Exp = mybir.ActivationFunctionType.Exp

    nc = bacc.Bacc(
        "TRN2",
        target_bir_lowering=False,
        debug=False,
        enable_asserts=False,
        num_devices=NCORES,
    )

    xT = nc.dram_tensor("xT", [D, S], bf16, kind="ExternalInput").ap()
    qw = nc.dram_tensor("qw", [D, GD], bf16, kind="ExternalInput").ap()
    kw = nc.dram_tensor("kw", [D, GD], bf16, kind="ExternalInput").ap()
    vw = nc.dram_tensor("vw", [D, GD], bf16, kind="ExternalInput").ap()
    qbias = nc.dram_tensor("qbias", [GD, 1], f32, kind="ExternalInput").ap()
    kbias = nc.dram_tensor("kbias", [GD, 1], f32, kind="ExternalInput").ap()
    wp = nc.dram_tensor("wp", [GD, D], bf16, kind="ExternalInput").ap()
    w_out = nc.dram_tensor("w_out", [G, S, S], bf16, kind="ExternalOutput").ap()
    a_out = nc.dram_tensor("a_out", [S, D], f32, kind="ExternalOutput").ap()

    with tile.TileContext(nc) as tc, ExitStack() as ctx:
        const = ctx.enter_context(tc.tile_pool(name="const", bufs=1))
        small = ctx.enter_context(tc.tile_pool(name="small", bufs=4))
        psum = ctx.enter_context(tc.tile_pool(name="psum", bufs=2, space="PSUM"))
        xqkv = ctx.enter_context(tc.tile_pool(name="xqkv", bufs=1))

        # ---- persistent SBUF tensors (one big tile per input tensor so a
        # single mega-DMA can fill it; slices act as the per-128-row tiles)
        xT_big = xqkv.tile([128, DK * S], bf16, tag="xT", name="xT_big")
        qw_big = xqkv.tile([128, DK * GD], bf16, tag="qw", name="qw_big")
        kw_big = xqkv.tile([128, DK * GD], bf16, tag="kw", name="kw_big")
        vw_big = xqkv.tile([128, DK * GD], bf16, tag="vw", name="vw_big")
        wp_big = const.tile([128, FT * D], bf16, tag="wp", name="wp_big")
        qb_big = const.tile([128, FT], f32, tag="qb", name="qb_big")
        kb_big = const.tile([128, FT], f32, tag="kb", name="kb_big")
        xT_sb = [xT_big[:, S * i:S * (i + 1)] for i in range(DK)]
        qw_sb = [qw_big[:, GD * i:GD * (i + 1)] for i in range(DK)]
        kw_sb = [kw_big[:, GD * i:GD * (i + 1)] for i in range(DK)]
        vw_sb = [vw_big[:, GD * i:GD * (i + 1)] for i in range(DK)]
        wp_sb = [wp_big[:, D * i:D * (i + 1)] for i in range(FT)]
        qb_sb = [qb_big[:, i:i + 1] for i in range(FT)]
        kb_sb = [kb_big[:, i:i + 1] for i in range(FT)]
        qT_sb = [const.tile([128, S], bf16, tag=f"qT{i}", name=f"qT{i}") for i in range(FT)]
        kT_sb = [const.tile([128, S], bf16, tag=f"kT{i}", name=f"kT{i}") for i in range(FT)]
        v_sb = [const.tile([128, G * VW], bf16, tag=f"v{i}", name=f"v{i}") for i in range(NB)]
        ident = const.tile([128, 128], bf16, tag="ident")

        # one mega-DMA per input tensor (fans across all DMA engines and
        # avoids per-dma_start issue serialization on the sequencer);
        # q/k path first (scores need them before v).
        def load_big(big, dram, n, nt):
            nc.sync.dma_start(
                big[:].rearrange("p (a s) -> p a s", a=nt),
                dram.rearrange("(a p) s -> p a s", p=128),
            )

        load_big(xT_big, xT, S, DK)
        load_big(qw_big, qw, GD, DK)
        load_big(kw_big, kw, GD, DK)
        load_big(qb_big, qbias, 1, FT)
        load_big(kb_big, kbias, 1, FT)
        load_big(vw_big, vw, GD, DK)
        load_big(wp_big, wp, D, FT)
        make_identity(nc, ident[:])

        # PE warm-up: the HAM clock gate keeps the PE at 1.2 GHz until it
        # has seen ~3.4us of sustained activity.  Dummy matmuls on scratch
        # SBUF (no DMA deps -> issue from t=0) span the input-load phase so
        # the real QKV matmuls start at 2.4 GHz.
        scratch = const.tile([128, 512], bf16, tag="scratch")
        nc.gpsimd.memset(scratch[:], 0.0)
        for i in range(28):
            ps_d = psum.tile([128, 1024], f32, tag="ps_wide", name="ps_d")
            nc.tensor.matmul(
                ps_d[:, 0:512], scratch[:, 0:128], scratch[:],
                start=True, stop=True,
            )

        # ---- QKV projections ----
        # qT/kT in [feature, token] layout: psum = w.T @ xT; bias on DVE.
        for ft in range(FT):
            for nh in range(2):
                psq = psum.tile([128, 512], f32, tag="ps_t", name="psq")
                for dk in range(DK):
                    nc.tensor.matmul(
                        psq[:], qw_sb[dk][:, ts(ft, 128)], xT_sb[dk][:, ts(nh, 512)],
                        start=(dk == 0), stop=(dk == DK - 1),
                    )
                nc.vector.tensor_scalar_add(
                    qT_sb[ft][:, ts(nh, 512)], psq[:], qb_sb[ft][:]
                )
                psk = psum.tile([128, 512], f32, tag="ps_t", name="psk")
                for dk in range(DK):
                    nc.tensor.matmul(
                        psk[:], kw_sb[dk][:, ts(ft, 128)], xT_sb[dk][:, ts(nh, 512)],
                        start=(dk == 0), stop=(dk == DK - 1),
                    )
                nc.vector.tensor_scalar_add(
                    kT_sb[ft][:, ts(nh, 512)], psk[:], kb_sb[ft][:]
                )
        # v in [token, feature] layout with a ones column per head (the
        # ones column turns the A@V matmul into a free row-sum producer).
        # v bias is folded on the host.  Emission is deferred: the v chains
        # are dense PE work slotted into the first pair's phase-1 latency.
        def emit_v(tt):
            psv = psum.tile([128, 512], f32, tag="ps_t", name="psv")
            for dk in range(DK):
                nc.tensor.matmul(
                    psv[:], xT_sb[dk][:, ts(tt, 128)], vw_sb[dk][:],
                    start=(dk == 0), stop=(dk == DK - 1),
                )
            vv = v_sb[tt][:].rearrange("p (h e) -> p h e", h=G)
            nc.vector.tensor_copy(
                vv[:, :, 0:HD], psv[:].rearrange("p (h d) -> p h d", h=G)
            )
            nc.vector.memset(vv[:, :, HD:VW], 1.0)

        work = ctx.enter_context(tc.tile_pool(name="work", bufs=3))

        # ---- attention: qb outer so the output projection of row-block
        # qb overlaps the attention of qb+1.  Within a qb, all 8 heads'
        # score rows + exps run first (phase 1) and all A@V chains second
        # (phase 2), so the ACT-exp / GpSimd-mask latency of head h hides
        # behind the score matmuls of heads h+1.. instead of stalling PE.
        def emit_transposes(attn_prev):
            aTt = []
            for ft in range(FT):
                ps_tr = psum.tile([128, 1024], f32, tag="ps_t", name="ps_tr")
                nc.tensor.matmul(
                    ps_tr[:, 0:128], attn_prev[:, ts(ft, 128)], ident[:],
                    start=True, stop=True,
                )
                t = work.tile([128, 128], bf16, tag=f"aTt{ft}", name="aTt", bufs=5)
                nc.scalar.copy(t[:], ps_tr[:, 0:128])
                aTt.append(t)
            return aTt

        def emit_proj(aTt, qb_prev):
            ps_p = psum.tile([128, 1024], f32, tag="ps_t", name="ps_p")
            for nh in range(2):
                for ft in range(FT):
                    nc.tensor.matmul(
                        ps_p[:, ts(nh, 512)], aTt[ft][:], wp_sb[ft][:, ts(nh, 512)],
                        start=(ft == 0), stop=(ft == FT - 1),
                    )
            ao = work.tile([128, 1024], f32, tag="aout", name="ao")
            nc.scalar.copy(ao[:], ps_p[:])
            nc.sync.dma_start(a_out[ts(qb_prev, 128), :], ao[:])

        # Row-blocks are processed in PAIRS (qb0, qb1=qb0+1): the
        # transposed-score matmuls cover both blocks' q columns at once
        # (N=256 instead of N=128 -- half the PE instruction overhead), at
        # the cost of one wasted 128x128 exp tile per (pair, head).
        # Heads are processed in GROUPS of 4, software-pipelined: the A@V /
        # normalize phase of group i runs while the score/exp phase of
        # group i+1 occupies the PE, so exp+mask latency never stalls it.
        def emit_phase1(pc, h):
            qb0, qb1, W0, W1, NKB = pc["qb0"], pc["qb1"], pc["W0"], pc["W1"], pc["NKB"]
            fq = h // 2
            po = 64 * (h % 2)
            kTh = kT_sb[fq][po:po + 64, :]
            qsl2 = qT_sb[fq][po:po + 64, qb0 * 128:qb0 * 128 + 256]

            # score rows s[q, k] for both blocks
            E2 = []
            for qb, W in ((qb0, W0), (qb1, W1)):
                ps_s = psum.tile([128, 1024], f32, tag="ps_wide", name="ps_s")
                qsl = qT_sb[fq][po:po + 64, ts(qb, 128)]
                for c0 in range(0, W, 512):
                    cw = min(512, W - c0)
                    nc.tensor.matmul(
                        ps_s[:, c0:c0 + cw], qsl, kTh[:, c0:c0 + cw],
                        start=True, stop=True,
                    )
                E = work.tile([128, S], bf16, tag="E", name="E", bufs=20)
                nc.scalar.activation(E[:, :W], ps_s[:, :W], Exp)
                # zero the masked (k > q) part of the diagonal block
                nc.gpsimd.affine_select(
                    out=E[:, ts(qb, 128)], in_=E[:, ts(qb, 128)],
                    compare_op=mybir.AluOpType.is_ge, fill=0.0,
                    base=0, pattern=[[-1, 128]], channel_multiplier=1,
                )
                E2.append(E)
            pc["Es"][h] = E2

            # transposed scores sT[k, q0|q1] -> exp -> ET (unnormalized)
            # layout: ET[:, kb*256 + 0:128] is (kb, qb0), +128:256 is (kb, qb1)
            ET = work.tile([128, 2048], bf16, tag="ET", name="ET", bufs=10)
            for k0 in range(0, NKB, 4):
                kn = min(4, NKB - k0)
                ps_t = psum.tile([128, 1024], f32, tag="ps_t", name="ps_t")
                for j in range(kn):
                    kb = k0 + j
                    nc.tensor.matmul(
                        ps_t[:, j * 256:(j + 1) * 256],
                        kTh[:, ts(kb, 128)], qsl2,
                        start=True, stop=True,
                    )
                nc.scalar.activation(
                    ET[:, k0 * 256:(k0 + kn) * 256], ps_t[:, :kn * 256], Exp
                )
            # diagonal-block masks: (qb0, qb0) at col 0, (qb1, qb1) at col 128
            nc.gpsimd.affine_select(
                out=ET[:, qb0 * 256:qb0 * 256 + 128],
                in_=ET[:, qb0 * 256:qb0 * 256 + 128],
                compare_op=mybir.AluOpType.is_ge, fill=0.0,
                base=0, pattern=[[1, 128]], channel_multiplier=-1,
            )
            nc.gpsimd.affine_select(
                out=ET[:, qb1 * 256 + 128:qb1 * 256 + 256],
                in_=ET[:, qb1 * 256 + 128:qb1 * 256 + 256],
                compare_op=mybir.AluOpType.is_ge, fill=0.0,
                base=0, pattern=[[1, 128]], channel_multiplier=-1,
            )
            pc["ETs"][h] = ET

        def emit_phase2(pc, h):
            qb0, qb1, W0, W1 = pc["qb0"], pc["qb1"], pc["W0"], pc["W1"]
            ET = pc["ETs"][h]
            for qi, (qb, W, attn_t) in enumerate(
                ((qb0, W0, pc["attn0"]), (qb1, W1, pc["attn1"]))
            ):
                E = pc["Es"][h][qi]
                # a[q, 0:64] = ET.T @ v ; a[q, 64] = row-sum (ones col)
                ps_a = psum.tile([128, VW], f32, tag="ps_wide", name="ps_a")
                for kb in range(qb + 1):
                    nc.tensor.matmul(
                        ps_a[:], ET[:, kb * 256 + qi * 128:kb * 256 + qi * 128 + 128],
                        v_sb[kb][:, VW * h:VW * (h + 1)],
                        start=(kb == 0), stop=(kb == qb),
                    )
                rcp = small.tile([128, 1], f32, tag="rcp", name="rcp")
                nc.vector.reciprocal(rcp[:], ps_a[:, HD:VW])

                wrow = work.tile([128, S], bf16, tag="wrow", name="wrow")
                nc.vector.tensor_scalar_mul(wrow[:, :W], E[:, :W], rcp[:])
                nc.sync.dma_start(w_out[h, ts(qb, 128), 0:W], wrow[:, :W])
                nc.vector.tensor_scalar_mul(
                    attn_t[:, HD * h:HD * (h + 1)], ps_a[:, 0:HD], rcp[:]
                )

        # per pair, per 4-head group: phase1 then phase2; previous pair's
        # transposes after the first phase1, its projection at pair end.
        prev_pc = None
        prev_aTt = None
        for qg in reversed(range(NB // 2)):
            qb0, qb1 = 2 * qg, 2 * qg + 1
            pc = {
                "qb0": qb0, "qb1": qb1,
                "W0": (qb0 + 1) * 128, "W1": (qb1 + 1) * 128,
                "NKB": qb1 + 1,
                "attn0": work.tile([128, GD], bf16, tag="attn", name="attn0", bufs=4),
                "attn1": work.tile([128, GD], bf16, tag="attn", name="attn1", bufs=4),
                "Es": {}, "ETs": {},
            }
            for h in range(G):
                emit_phase1(pc, h)
            if prev_pc is None:
                # first pair: the v-projection chains fill the PE here
                for tt in range(NB):
                    emit_v(tt)
            else:
                prev_aTt = [
                    emit_transposes(prev_pc["attn0"]),
                    emit_transposes(prev_pc["attn1"]),
                ]
            for h in range(G):
                emit_phase2(pc, h)
            if prev_pc is not None:
                emit_proj(prev_aTt[0], prev_pc["qb0"])
                emit_proj(prev_aTt[1], prev_pc["qb1"])
            prev_pc = pc

        emit_proj(emit_transposes(prev_pc["attn0"]), prev_pc["qb0"])
        emit_proj(emit_transposes(prev_pc["attn1"]), prev_pc["qb1"])

    nc.compile()
    return nc


def _numpy_reference(x, mask, w_attn, b_attn, w_proj, b_proj):
    # Defensive fallback for a nonzero padding mask (the problem spec fills
    # it with zeros); replicates the reference math in f32 numpy.
    b, s, d = x.shape
    qkv = x @ w_attn + b_attn
    q, k, v = np.split(qkv, 3, axis=2)

    def split_heads(t):
        return t.reshape(b, s, H, HD).transpose(0, 2, 1, 3)

    q, k, v = split_heads(q), split_heads(k), split_heads(v)
    w = np.einsum("bhqd,bhkd->bhqk", q, k) / np.sqrt(np.float32(HD))
    i = np.arange(s)[:, None]
    j = np.arange(s)[None, :]
    bmask = (i >= j).astype(w.dtype)[None, None]
    w = w * bmask - NEG * (1.0 - bmask)
    w = w + mask
    w = w - w.max(axis=-1, keepdims=True)
    e = np.exp(w)
    w = e / e.sum(axis=-1, keepdims=True)
    a = np.einsum("bhqk,bhkd->bhqd", w, v)
    a = a.transpose(0, 2, 1, 3).reshape(b, s, d)
    a = a @ w_proj + b_proj
    return a.astype(np.float32), w.astype(np.float32)


def kernel(**inputs):
    x = np.asarray(inputs["x"], dtype=np.float32)
    mask = np.asarray(inputs["mask"], dtype=np.float32)
    w_attn = np.asarray(inputs["w_attn"], dtype=np.float32)
    b_attn = np.asarray(inputs["b_attn"], dtype=np.float32)
    w_proj = np.asarray(inputs["w_proj"], dtype=np.float32)
    b_proj = np.asarray(inputs["b_proj"], dtype=np.float32)

    if np.any(mask != 0.0):
        return _numpy_reference(x, mask, w_attn, b_attn, w_proj, b_proj)

    from concourse.bass_utils import run_bass_kernel_spmd

    if "nc" not in _CACHE:
        _CACHE["nc"] = _build_bass()
    nc = _CACHE["nc"]

    bf = ml_dtypes.bfloat16
    in_maps = []
    for c in range(NCORES):
        b, g = divmod(c, 2)
        sl = slice(GD * g, GD * (g + 1))
        in_maps.append({
            "xT": np.ascontiguousarray(x[b].T).astype(bf),
            "qw": np.ascontiguousarray(w_attn[:, sl] * 0.125).astype(bf),
            "kw": np.ascontiguousarray(w_attn[:, D + GD * g:D + GD * (g + 1)]).astype(bf),
            "vw": np.ascontiguousarray(w_attn[:, 2 * D + GD * g:2 * D + GD * (g + 1)]).astype(bf),
            "qbias": np.ascontiguousarray(b_attn[0, sl] * 0.125).reshape(GD, 1).astype(np.float32),
            "kbias": np.ascontiguousarray(b_attn[0, D + GD * g:D + GD * (g + 1)]).reshape(GD, 1).astype(np.float32),
            "wp": np.ascontiguousarray(w_proj[sl, :]).astype(bf),
        })

    res = run_bass_kernel_spmd(nc, in_maps, core_ids=list(range(NCORES))).results

    w = np.empty((B, H, S, S), np.float32)
    a = np.empty((B, S, D), np.float32)
    for c in range(NCORES):
        b, g = divmod(c, 2)
        w[b, G * g:G * (g + 1)] = res[c]["w_out"].astype(np.float32)
    for b in range(B):
        a[b] = res[2 * b]["a_out"] + res[2 * b + 1]["a_out"]
    # exact host-side bias folds: v-bias contributes (sum_k w = 1) b_v @ w_proj
    a += (b_proj[0] + b_attn[0, 2 * D:] @ w_proj).reshape(1, 1, D)
    return a, w


# revision 38
# speedup vs baseline: 1.0192x; 1.0192x over previous
"""Distributed causal-attention kernel for 8 TRN2 NeuronCores.

Problem: B=4, S=1024, D=1024, H=16 causal attention block returning
(a, w) where a = proj(attn output) and w = softmax attention probs.

Sharding (tensor-parallel heads x data-parallel batch):
  core c -> (batch b = c//2, head-group g = c%2) ; each group = 8 heads.
  Each core computes its group's QKV projection, causal softmax
  (writing its [8, S, S] slice of w), A@V, and a partial output
  projection a_part = attn_g @ w_proj[rows g].  Host sums the two
  partials per batch (a = part0 + part1 + b_proj + b_attn_v @ w_proj).
  No device collectives needed.

Compute is bf16 on the TensorEngine (f32 PSUM accumulation); softmax
exp runs in f32 on the ScalarEngine.  The 1/sqrt(HD)=0.125 score scale
is folded into the Q weights on the host (exact, power of two).
Scores are computed in both [q,k] (for the softmax/w output) and
[k,q] (for the A@V matmul) orientations -- cheaper on TRN2 than any
transpose path.  Causal masking zeroes the diagonal-block upper
triangle on GpSimd after the exp; softmax row-sums ride along as a
ones-column in the A@V matmul, so the masked upper region is never
computed.  Upper-triangular w stays zero: output DRAM buffers are
zero-initialized by the runtime, so that region is never written.
"""

import numpy as np
import ml_dtypes

B, S, D, H = 4, 1024, 1024, 16
HD = D // H          # 64 head dim
G = H // 2           # 8 heads per core
GD = G * HD          # 512 dims per head group
VW = HD + 1          # v columns per head incl. ones column
NB = S // 128        # 8 token blocks
DK = D // 128        # 8 contraction tiles
FT = GD // 128       # 4 feature tiles per group
NCORES = 8
NEG = 10000.0

_CACHE = {}


def _build_bass():
    from contextlib import ExitStack
    import concourse.bass as bass
    import concourse.tile as tile
    from concourse import bacc, mybir
    from concourse.masks import make_identity

    f32 = mybir.dt.float32
    bf16 = mybir.dt.bfloat16
    ts = bass.ts
    Exp = mybir.ActivationFunctionType.Exp

    nc = bacc.Bacc(
        "TRN2",
        target_bir_lowering=False,
        debug=False,
        enable_asserts=False,
        num_devices=NCORES,
    )

    xT = nc.dram_tensor("xT", [D, S], bf16, kind="ExternalInput").ap()
    qw = nc.dram_tensor("qw", [D, GD], bf16, kind="ExternalInput").ap()
    kw = nc.dram_tensor("kw", [D, GD], bf16, kind="ExternalInput").ap()
    vw = nc.dram_tensor("vw", [D, GD], bf16, kind="ExternalInput").ap()
    qbias = nc.dram_tensor("qbias", [GD, 1], f32, kind="ExternalInput").ap()
    kbias = nc.dram_tensor("kbias", [GD, 1], f32, kind="ExternalInput").ap()
    wp = nc.dram_tensor("wp", [GD, D], bf16, kind="ExternalInput").ap()
    w_out = nc.dram_tensor("w_out", [G, S, S], bf16, kind="ExternalOutput").ap()
    a_out = nc.dram_tensor("a_out", [S, D], f32, kind="ExternalOutput").ap()

    with tile.TileContext(nc) as tc, ExitStack() as ctx:
        const = ctx.enter_context(tc.tile_pool(name="const", bufs=1))
        small = ctx.enter_context(tc.tile_pool(name="small", bufs=4))
        psum = ctx.enter_context(tc.tile_pool(name="psum", bufs=2, space="PSUM"))
        xqkv = ctx.enter_context(tc.tile_pool(name="xqkv", bufs=1))

        # ---- persistent SBUF tensors (one big tile per input tensor so a
        # single mega-DMA can fill it; slices act as the per-128-row tiles)
        xT_big = xqkv.tile([128, DK * S], bf16, tag="xT", name="xT_big")
        qw_big = xqkv.tile([128, DK * GD], bf16, tag="qw", name="qw_big")
        kw_big = xqkv.tile([128, DK * GD], bf16, tag="kw", name="kw_big")
        vw_big = xqkv.tile([128, DK * GD], bf16, tag="vw", name="vw_big")
        wp_big = const.tile([128, FT * D], bf16, tag="wp", name="wp_big")
        qb_big = const.tile([128, FT], f32, tag="qb", name="qb_big")
        kb_big = const.tile([128, FT], f32, tag="kb", name="kb_big")
        xT_sb = [xT_big[:, S * i:S * (i + 1)] for i in range(DK)]
        qw_sb = [qw_big[:, GD * i:GD * (i + 1)] for i in range(DK)]
        kw_sb = [kw_big[:, GD * i:GD * (i + 1)] for i in range(DK)]
        vw_sb = [vw_big[:, GD * i:GD * (i + 1)] for i in range(DK)]
        wp_sb = [wp_big[:, D * i:D * (i + 1)] for i in range(FT)]
        qb_sb = [qb_big[:, i:i + 1] for i in range(FT)]
        kb_sb = [kb_big[:, i:i + 1] for i in range(FT)]
        qT_sb = [const.tile([128, S], bf16, tag=f"qT{i}", name=f"qT{i}") for i in range(FT)]
        kT_sb = [const.tile([128, S], bf16, tag=f"kT{i}", name=f"kT{i}") for i in range(FT)]
        v_sb = [const.tile([128, G * VW], bf16, tag=f"v{i}", name=f"v{i}") for i in range(NB)]
        ident = const.tile([128, 128], bf16, tag="ident")

        # one mega-DMA per input tensor (fans across all DMA engines and
        # avoids per-dma_start issue serialization on the sequencer);
        # q/k path first (scores need them before v).
        def load_big(big, dram, n, nt):
            nc.sync.dma_start(
                big[:].rearrange("p (a s) -> p a s", a=nt),
                dram.rearrange("(a p) s -> p a s", p=128),
            )

        load_big(xT_big, xT, S, DK)
        load_big(qw_big, qw, GD, DK)
        load_big(kw_big, kw, GD, DK)
        load_big(qb_big, qbias, 1, FT)
        load_big(kb_big, kbias, 1, FT)
        load_big(vw_big, vw, GD, DK)
        load_big(wp_big, wp, D, FT)
        make_identity(nc, ident[:])

        # PE warm-up: the HAM clock gate keeps the PE at 1.2 GHz until it
        # has seen ~3.4us of sustained activity.  Dummy matmuls on scratch
        # SBUF (no DMA deps -> issue from t=0) span the input-load phase so
        # the real QKV matmuls start at 2.4 GHz.
        scratch = const.tile([128, 512], bf16, tag="scratch")
        nc.gpsimd.memset(scratch[:], 0.0)
        for i in range(28):
            ps_d = psum.tile([128, 1024], f32, tag="ps_wide", name="ps_d")
            nc.tensor.matmul(
                ps_d[:, 0:512], scratch[:, 0:128], scratch[:],
                start=True, stop=True,
            )

        # ---- QKV projections ----
        # qT/kT in [feature, token] layout: psum = w.T @ xT; bias on DVE.
        for ft in range(FT):
            for nh in range(2):
                psq = psum.tile([128, 512], f32, tag="ps_t", name="psq")
                for dk in range(DK):
                    nc.tensor.matmul(
                        psq[:], qw_sb[dk][:, ts(ft, 128)], xT_sb[dk][:, ts(nh, 512)],
                        start=(dk == 0), stop=(dk == DK - 1),
                    )
                nc.vector.tensor_scalar_add(
                    qT_sb[ft][:, ts(nh, 512)], psq[:], qb_sb[ft][:]
                )
                psk = psum.tile([128, 512], f32, tag="ps_t", name="psk")
                for dk in range(DK):
                    nc.tensor.matmul(
                        psk[:], kw_sb[dk][:, ts(ft, 128)], xT_sb[dk][:, ts(nh, 512)],
                        start=(dk == 0), stop=(dk == DK - 1),
                    )
                nc.vector.tensor_scalar_add(
                    kT_sb[ft][:, ts(nh, 512)], psk[:], kb_sb[ft][:]
                )
        # v in [token, feature] layout with a ones column per head (the
        # ones column turns the A@V matmul into a free row-sum producer).
        # v bias is folded on the host.  Emission is deferred: the v chains
        # are dense PE work slotted into the first pair's phase-1 latency.
        def emit_v(tt):
            psv = psum.tile([128, 512], f32, tag="ps_t", name="psv")
            for dk in range(DK):
                nc.tensor.matmul(
                    psv[:], xT_sb[dk][:, ts(tt, 128)], vw_sb[dk][:],
                    start=(dk == 0), stop=(dk == DK - 1),
                )
            vv = v_sb[tt][:].rearrange("p (h e) -> p h e", h=G)
            nc.vector.tensor_copy(
                vv[:, :, 0:HD], psv[:].rearrange("p (h d) -> p h d", h=G)
            )
            nc.vector.memset(vv[:, :, HD:VW], 1.0)

        work = ctx.enter_context(tc.tile_pool(name="work", bufs=3))

        # ---- attention: qb outer so the output projection of row-block
        # qb overlaps the attention of qb+1.  Within a qb, all 8 heads'
        # score rows + exps run first (phase 1) and all A@V chains second
        # (phase 2), so the ACT-exp / GpSimd-mask latency of head h hides
        # behind the score matmuls of heads h+1.. instead of stalling PE.
        def emit_transposes(attn_prev):
            aTt = []
            for ft in range(FT):
                ps_tr = psum.tile([128, 1024], f32, tag="ps_t", name="ps_tr")
                nc.tensor.matmul(
                    ps_tr[:, 0:128], attn_prev[:, ts(ft, 128)], ident[:],
                    start=True, stop=True,
                )
                t = work.tile([128, 128], bf16, tag=f"aTt{ft}", name="aTt", bufs=5)
                nc.scalar.copy(t[:], ps_tr[:, 0:128])
                aTt.append(t)
            return aTt

        def emit_proj(aTt, qb_prev):
            ps_p = psum.tile([128, 1024], f32, tag="ps_t", name="ps_p")
            for nh in range(2):
                for ft in range(FT):
                    nc.tensor.matmul(
                        ps_p[:, ts(nh, 512)], aTt[ft][:], wp_sb[ft][:, ts(nh, 512)],
                        start=(ft == 0), stop=(ft == FT - 1),
                    )
            ao = work.tile([128, 1024], f32, tag="aout", name="ao")
            nc.scalar.copy(ao[:], ps_p[:])
            nc.sync.dma_start(a_out[ts(qb_prev, 128), :], ao[:])

        # Row-blocks are processed in PAIRS (qb0, qb1=qb0+1): the
        # transposed-score matmuls cover both blocks' q columns at once
        # (N=256 instead of N=128 -- half the PE instruction overhead), at
        # the cost of one wasted 128x128 exp tile per (pair, head).
        # Heads are processed in GROUPS of 4, software-pipelined: the A@V /
        # normalize phase of group i runs while the score/exp phase of
        # group i+1 occupies the PE, so exp+mask latency never stalls it.
        def emit_phase1(pc, h):
            qb0, qb1, W0, W1, NKB = pc["qb0"], pc["qb1"], pc["W0"], pc["W1"], pc["NKB"]
            fq = h // 2
            po = 64 * (h % 2)
            kTh = kT_sb[fq][po:po + 64, :]
            qsl2 = qT_sb[fq][po:po + 64, qb0 * 128:qb0 * 128 + 256]

            # transposed scores sT[k, q0|q1] -> exp -> ET (unnormalized)
            # layout: ET[:, kb*256 + 0:128] is (kb, qb0), +128:256 is (kb, qb1)
            ET = work.tile([128, 2048], bf16, tag="ET", name="ET", bufs=6)
            for k0 in range(0, NKB, 4):
                kn = min(4, NKB - k0)
                ps_t = psum.tile([128, 1024], f32, tag="ps_t", name="ps_t")
                for j in range(kn):
                    kb = k0 + j
                    nc.tensor.matmul(
                        ps_t[:, j * 256:(j + 1) * 256],
                        kTh[:, ts(kb, 128)], qsl2,
                        start=True, stop=True,
                    )
                nc.scalar.activation(
                    ET[:, k0 * 256:(k0 + kn) * 256], ps_t[:, :kn * 256], Exp
                )
            # diagonal-block masks: (qb0, qb0) at col 0, (qb1, qb1) at col 128
            nc.gpsimd.affine_select(
                out=ET[:, qb0 * 256:qb0 * 256 + 128],
                in_=ET[:, qb0 * 256:qb0 * 256 + 128],
                compare_op=mybir.AluOpType.is_ge, fill=0.0,
                base=0, pattern=[[1, 128]], channel_multiplier=-1,
            )
            nc.gpsimd.affine_select(
                out=ET[:, qb1 * 256 + 128:qb1 * 256 + 256],
                in_=ET[:, qb1 * 256 + 128:qb1 * 256 + 256],
                compare_op=mybir.AluOpType.is_ge, fill=0.0,
                base=0, pattern=[[1, 128]], channel_multiplier=-1,
            )
            pc["ETs"][h] = ET

            # score rows s[q, k] for both blocks
            E2 = []
            for qb, W in ((qb0, W0), (qb1, W1)):
                ps_s = psum.tile([128, 1024], f32, tag="ps_wide", name="ps_s")
                qsl = qT_sb[fq][po:po + 64, ts(qb, 128)]
                for c0 in range(0, W, 512):
                    cw = min(512, W - c0)
                    nc.tensor.matmul(
                        ps_s[:, c0:c0 + cw], qsl, kTh[:, c0:c0 + cw],
                        start=True, stop=True,
                    )
                E = work.tile([128, S], bf16, tag="E", name="E", bufs=12)
                nc.scalar.activation(E[:, :W], ps_s[:, :W], Exp)
                # zero the masked (k > q) part of the diagonal block
                nc.gpsimd.affine_select(
                    out=E[:, ts(qb, 128)], in_=E[:, ts(qb, 128)],
                    compare_op=mybir.AluOpType.is_ge, fill=0.0,
                    base=0, pattern=[[-1, 128]], channel_multiplier=1,
                )
                E2.append(E)
            pc["Es"][h] = E2

        def emit_phase2(pc, h):
            qb0, qb1, W0, W1 = pc["qb0"], pc["qb1"], pc["W0"], pc["W1"]
            ET = pc["ETs"][h]
            for qi, (qb, W, attn_t) in enumerate(
                ((qb0, W0, pc["attn0"]), (qb1, W1, pc["attn1"]))
            ):
                E = pc["Es"][h][qi]
                # a[q, 0:64] = ET.T @ v ; a[q, 64] = row-sum (ones col)
                ps_a = psum.tile([128, VW], f32, tag="ps_wide", name="ps_a")
                for kb in range(qb + 1):
                    nc.tensor.matmul(
                        ps_a[:], ET[:, kb * 256 + qi * 128:kb * 256 + qi * 128 + 128],
                        v_sb[kb][:, VW * h:VW * (h + 1)],
                        start=(kb == 0), stop=(kb == qb),
                    )
                rcp = small.tile([128, 1], f32, tag="rcp", name="rcp")
                nc.vector.reciprocal(rcp[:], ps_a[:, HD:VW])

                wrow = work.tile([128, S], bf16, tag="wrow", name="wrow")
                nc.vector.tensor_scalar_mul(wrow[:, :W], E[:, :W], rcp[:])
                nc.sync.dma_start(w_out[h, ts(qb, 128), 0:W], wrow[:, :W])
                nc.vector.tensor_scalar_mul(
                    attn_t[:, HD * h:HD * (h + 1)], ps_a[:, 0:HD], rcp[:]
                )

        # per pair, per 4-head group: phase1 then phase2; previous pair's
        # transposes after the first phase1, its projection at pair end.
        prev_pc = None
        prev_aTt = None
        for qg in reversed(range(NB // 2)):
            qb0, qb1 = 2 * qg, 2 * qg + 1
            pc = {
                "qb0": qb0, "qb1": qb1,
                "W0": (qb0 + 1) * 128, "W1": (qb1 + 1) * 128,
                "NKB": qb1 + 1,
                "attn0": work.tile([128, GD], bf16, tag="attn", name="attn0", bufs=4),
                "attn1": work.tile([128, GD], bf16, tag="attn", name="attn1", bufs=4),
                "Es": {}, "ETs": {},
            }
            for hg in range(2):
                for h in range(4 * hg, 4 * hg + 4):
                    emit_phase1(pc, h)
                if hg == 0 and prev_pc is None:
                    # first pair: the v-projection chains fill the PE here
                    for tt in range(NB):
                        emit_v(tt)
                if hg == 0 and prev_pc is not None:
                    prev_aTt = [
                        emit_transposes(prev_pc["attn0"]),
                        emit_transposes(prev_pc["attn1"]),
                    ]
                for h in range(4 * hg, 4 * hg + 4):
                    emit_phase2(pc, h)
            if prev_pc is not None:
                emit_proj(prev_aTt[0], prev_pc["qb0"])
                emit_proj(prev_aTt[1], prev_pc["qb1"])
            prev_pc = pc

        emit_proj(emit_transposes(prev_pc["attn0"]), prev_pc["qb0"])
        emit_proj(emit_transposes(prev_pc["attn1"]), prev_pc["qb1"])

    nc.compile()
    return nc


def _numpy_reference(x, mask, w_attn, b_attn, w_proj, b_proj):
    # Defensive fallback for a nonzero padding mask (the problem spec fills
    # it with zeros); replicates the reference math in f32 numpy.
    b, s, d = x.shape
    qkv = x @ w_attn + b_attn
    q, k, v = np.split(qkv, 3, axis=2)

    def split_heads(t):
        return t.reshape(b, s, H, HD).transpose(0, 2, 1, 3)

    q, k, v = split_heads(q), split_heads(k), split_heads(v)
    w = np.einsum("bhqd,bhkd->bhqk", q, k) / np.sqrt(np.float32(HD))
    i = np.arange(s)[:, None]
    j = np.arange(s)[None, :]
    bmask = (i >= j).astype(w.dtype)[None, None]
    w = w * bmask - NEG * (1.0 - bmask)
    w = w + mask
    w = w - w.max(axis=-1, keepdims=True)
    e = np.exp(w)
    w = e / e.sum(axis=-1, keepdims=True)
    a = np.einsum("bhqk,bhkd->bhqd", w, v)
    a = a.transpose(0, 2, 1, 3).reshape(b, s, d)
    a = a @ w_proj + b_proj
    return a.astype(np.float32), w.astype(np.float32)


def kernel(**inputs):
    x = np.asarray(inputs["x"], dtype=np.float32)
    mask = np.asarray(inputs["mask"], dtype=np.float32)
    w_attn = np.asarray(inputs["w_attn"], dtype=np.float32)
    b_attn = np.asarray(inputs["b_attn"], dtype=np.float32)
    w_proj = np.asarray(inputs["w_proj"], dtype=np.float32)
    b_proj = np.asarray(inputs["b_proj"], dtype=np.float32)

    if np.any(mask != 0.0):
        return _numpy_reference(x, mask, w_attn, b_attn, w_proj, b_proj)

    from concourse.bass_utils import run_bass_kernel_spmd

    if "nc" not in _CACHE:
        _CACHE["nc"] = _build_bass()
    nc = _CACHE["nc"]

    bf = ml_dtypes.bfloat16
    in_maps = []
    for c in range(NCORES):
        b, g = divmod(c, 2)
        sl = slice(GD * g, GD * (g + 1))
        in_maps.append({
            "xT": np.ascontiguousarray(x[b].T).astype(bf),
            "qw": np.ascontiguousarray(w_attn[:, sl] * 0.125).astype(bf),
            "kw": np.ascontiguousarray(w_attn[:, D + GD * g:D + GD * (g + 1)]).astype(bf),
            "vw": np.ascontiguousarray(w_attn[:, 2 * D + GD * g:2 * D + GD * (g + 1)]).astype(bf),
            "qbias": np.ascontiguousarray(b_attn[0, sl] * 0.125).reshape(GD, 1).astype(np.float32),
            "kbias": np.ascontiguousarray(b_attn[0, D + GD * g:D + GD * (g + 1)]).reshape(GD, 1).astype(np.float32),
            "wp": np.ascontiguousarray(w_proj[sl, :]).astype(bf),
        })

    res = run_bass_kernel_spmd(nc, in_maps, core_ids=list(range(NCORES))).results

    w = np.empty((B, H, S, S), np.float32)
    a = np.empty((B, S, D), np.float32)
    for c in range(NCORES):
        b, g = divmod(c, 2)
        w[b, G * g:G * (g + 1)] = res[c]["w_out"].astype(np.float32)
    for b in range(B):
        a[b] = res[2 * b]["a_out"] + res[2 * b + 1]["a_out"]
    # exact host-side bias folds: v-bias contributes (sum_k w = 1) b_v @ w_proj
    a += (b_proj[0] + b_attn[0, 2 * D:] @ w_proj).reshape(1, 1, D)
    return a, w


# revision 40
# speedup vs baseline: 1.0480x; 1.0282x over previous
"""Distributed causal-attention kernel for 8 TRN2 NeuronCores.

Problem: B=4, S=1024, D=1024, H=16 causal attention block returning
(a, w) where a = proj(attn output) and w = softmax attention probs.

Sharding (tensor-parallel heads x data-parallel batch):
  core c -> (batch b = c//2, head-group g = c%2) ; each group = 8 heads.
  Each core computes its group's QKV projection, causal softmax
  (writing its [8, S, S] slice of w), A@V, and a partial output
  projection a_part = attn_g @ w_proj[rows g].  Host sums the two
  partials per batch (a = part0 + part1 + b_proj + b_attn_v @ w_proj).
  No device collectives needed.

Compute is bf16 on the TensorEngine (f32 PSUM accumulation); softmax
exp runs in f32 on the ScalarEngine.  The 1/sqrt(HD)=0.125 score scale
is folded into the Q weights on the host (exact, power of two).
Scores are computed in both [q,k] (for the softmax/w output) and
[k,q] (for the A@V matmul) orientations -- cheaper on TRN2 than any
transpose path.  Causal masking zeroes the diagonal-block upper
triangle on GpSimd after the exp; softmax row-sums ride along as a
ones-column in the A@V matmul, so the masked upper region is never
computed.  Upper-triangular w stays zero: output DRAM buffers are
zero-initialized by the runtime, so that region is never written.
"""

import numpy as np
import ml_dtypes

B, S, D, H = 4, 1024, 1024, 16
HD = D // H          # 64 head dim
G = H // 2           # 8 heads per core
GD = G * HD          # 512 dims per head group
VW = HD + 1          # v columns per head incl. ones column
NB = S // 128        # 8 token blocks
DK = D // 128        # 8 contraction tiles
FT = GD // 128       # 4 feature tiles per group
NCORES = 8
NEG = 10000.0

_CACHE = {}


def _build_bass():
    from contextlib import ExitStack
    import concourse.bass as bass
    import concourse.tile as tile
    from concourse import bacc, mybir
    from concourse.masks import make_identity

    f32 = mybir.dt.float32
    bf16 = mybir.dt.bfloat16
    ts = bass.ts
    Exp = mybir.ActivationFunctionType.Exp

    nc = bacc.Bacc(
        "TRN2",
        target_bir_lowering=False,
        debug=False,
        enable_asserts=False,
        num_devices=NCORES,
    )

    xT = nc.dram_tensor("xT", [D, S], bf16, kind="ExternalInput").ap()
    qw = nc.dram_tensor("qw", [D, GD], bf16, kind="ExternalInput").ap()
    kw = nc.dram_tensor("kw", [D, GD], bf16, kind="ExternalInput").ap()
    vw = nc.dram_tensor("vw", [D, GD], bf16, kind="ExternalInput").ap()
    qbias = nc.dram_tensor("qbias", [GD, 1], f32, kind="ExternalInput").ap()
    kbias = nc.dram_tensor("kbias", [GD, 1], f32, kind="ExternalInput").ap()
    wp = nc.dram_tensor("wp", [GD, D], bf16, kind="ExternalInput").ap()
    w_out = nc.dram_tensor("w_out", [G, S, S], bf16, kind="ExternalOutput").ap()
    a_out = nc.dram_tensor("a_out", [S, D], f32, kind="ExternalOutput").ap()

    with tile.TileContext(nc) as tc, ExitStack() as ctx:
        const = ctx.enter_context(tc.tile_pool(name="const", bufs=1))
        small = ctx.enter_context(tc.tile_pool(name="small", bufs=4))
        psum = ctx.enter_context(tc.tile_pool(name="psum", bufs=2, space="PSUM"))
        xqkv = ctx.enter_context(tc.tile_pool(name="xqkv", bufs=1))

        # ---- persistent SBUF tensors (one big tile per input tensor so a
        # single mega-DMA can fill it; slices act as the per-128-row tiles)
        xT_big = xqkv.tile([128, DK * S], bf16, tag="xT", name="xT_big")
        qw_big = xqkv.tile([128, DK * GD], bf16, tag="qw", name="qw_big")
        kw_big = xqkv.tile([128, DK * GD], bf16, tag="kw", name="kw_big")
        vw_big = xqkv.tile([128, DK * GD], bf16, tag="vw", name="vw_big")
        wp_big = const.tile([128, FT * D], bf16, tag="wp", name="wp_big")
        qb_big = const.tile([128, FT], f32, tag="qb", name="qb_big")
        kb_big = const.tile([128, FT], f32, tag="kb", name="kb_big")
        xT_sb = [xT_big[:, S * i:S * (i + 1)] for i in range(DK)]
        qw_sb = [qw_big[:, GD * i:GD * (i + 1)] for i in range(DK)]
        kw_sb = [kw_big[:, GD * i:GD * (i + 1)] for i in range(DK)]
        vw_sb = [vw_big[:, GD * i:GD * (i + 1)] for i in range(DK)]
        wp_sb = [wp_big[:, D * i:D * (i + 1)] for i in range(FT)]
        qb_sb = [qb_big[:, i:i + 1] for i in range(FT)]
        kb_sb = [kb_big[:, i:i + 1] for i in range(FT)]
        qT_sb = [const.tile([128, S], bf16, tag=f"qT{i}", name=f"qT{i}") for i in range(FT)]
        kT_sb = [const.tile([128, S], bf16, tag=f"kT{i}", name=f"kT{i}") for i in range(FT)]
        v_sb = [const.tile([128, G * VW], bf16, tag=f"v{i}", name=f"v{i}") for i in range(NB)]
        ident = const.tile([128, 128], bf16, tag="ident")

        # one mega-DMA per input tensor (fans across all DMA engines and
        # avoids per-dma_start issue serialization on the sequencer);
        # q/k path first (scores need them before v).
        def load_big(big, dram, n, nt):
            nc.sync.dma_start(
                big[:].rearrange("p (a s) -> p a s", a=nt),
                dram.rearrange("(a p) s -> p a s", p=128),
            )

        load_big(xT_big, xT, S, DK)
        load_big(qw_big, qw, GD, DK)
        load_big(kw_big, kw, GD, DK)
        load_big(qb_big, qbias, 1, FT)
        load_big(kb_big, kbias, 1, FT)
        load_big(vw_big, vw, GD, DK)
        load_big(wp_big, wp, D, FT)
        make_identity(nc, ident[:])

        # PE warm-up: the HAM clock gate keeps the PE at 1.2 GHz until it
        # has seen ~3.4us of sustained activity.  Dummy matmuls on scratch
        # SBUF (no DMA deps -> issue from t=0) span the input-load phase so
        # the real QKV matmuls start at 2.4 GHz.
        scratch = const.tile([128, 512], bf16, tag="scratch")
        nc.gpsimd.memset(scratch[:], 0.0)
        for i in range(28):
            ps_d = psum.tile([128, 1024], f32, tag="ps_wide", name="ps_d")
            nc.tensor.matmul(
                ps_d[:, 0:512], scratch[:, 0:128], scratch[:],
                start=True, stop=True,
            )

        # ---- QKV projections ----
        # qT/kT in [feature, token] layout: psum = w.T @ xT; bias on DVE.
        for ft in range(FT):
            for nh in range(2):
                psq = psum.tile([128, 512], f32, tag="ps_t", name="psq")
                for dk in range(DK):
                    nc.tensor.matmul(
                        psq[:], qw_sb[dk][:, ts(ft, 128)], xT_sb[dk][:, ts(nh, 512)],
                        start=(dk == 0), stop=(dk == DK - 1),
                    )
                nc.vector.tensor_scalar_add(
                    qT_sb[ft][:, ts(nh, 512)], psq[:], qb_sb[ft][:]
                )
                psk = psum.tile([128, 512], f32, tag="ps_t", name="psk")
                for dk in range(DK):
                    nc.tensor.matmul(
                        psk[:], kw_sb[dk][:, ts(ft, 128)], xT_sb[dk][:, ts(nh, 512)],
                        start=(dk == 0), stop=(dk == DK - 1),
                    )
                nc.vector.tensor_scalar_add(
                    kT_sb[ft][:, ts(nh, 512)], psk[:], kb_sb[ft][:]
                )
        # v in [token, feature] layout with a ones column per head (the
        # ones column turns the A@V matmul into a free row-sum producer).
        # v bias is folded on the host.  Emission is deferred: the v chains
        # are dense PE work slotted into the first pair's phase-1 latency.
        def emit_v(tt):
            psv = psum.tile([128, 512], f32, tag="ps_t", name="psv")
            for dk in range(DK):
                nc.tensor.matmul(
                    psv[:], xT_sb[dk][:, ts(tt, 128)], vw_sb[dk][:],
                    start=(dk == 0), stop=(dk == DK - 1),
                )
            vv = v_sb[tt][:].rearrange("p (h e) -> p h e", h=G)
            nc.vector.tensor_copy(
                vv[:, :, 0:HD], psv[:].rearrange("p (h d) -> p h d", h=G)
            )
            nc.vector.memset(vv[:, :, HD:VW], 1.0)

        work = ctx.enter_context(tc.tile_pool(name="work", bufs=3))

        # ---- attention: qb outer so the output projection of row-block
        # qb overlaps the attention of qb+1.  Within a qb, all 8 heads'
        # score rows + exps run first (phase 1) and all A@V chains second
        # (phase 2), so the ACT-exp / GpSimd-mask latency of head h hides
        # behind the score matmuls of heads h+1.. instead of stalling PE.
        def emit_transposes(attn_prev):
            aTt = []
            for ft in range(FT):
                ps_tr = psum.tile([128, 1024], f32, tag="ps_t", name="ps_tr")
                nc.tensor.matmul(
                    ps_tr[:, 0:128], attn_prev[:, ts(ft, 128)], ident[:],
                    start=True, stop=True,
                )
                t = work.tile([128, 128], bf16, tag=f"aTt{ft}", name="aTt", bufs=5)
                nc.scalar.copy(t[:], ps_tr[:, 0:128])
                aTt.append(t)
            return aTt

        def emit_proj(aTt, qb_prev):
            ps_p = psum.tile([128, 1024], f32, tag="ps_t", name="ps_p")
            for nh in range(2):
                for ft in range(FT):
                    nc.tensor.matmul(
                        ps_p[:, ts(nh, 512)], aTt[ft][:], wp_sb[ft][:, ts(nh, 512)],
                        start=(ft == 0), stop=(ft == FT - 1),
                    )
            ao = work.tile([128, 1024], f32, tag="aout", name="ao")
            nc.scalar.copy(ao[:], ps_p[:])
            nc.sync.dma_start(a_out[ts(qb_prev, 128), :], ao[:])

        # Row-blocks are processed in PAIRS (qb0, qb1=qb0+1): the
        # transposed-score matmuls cover both blocks' q columns at once
        # (N=256 instead of N=128 -- half the PE instruction overhead), at
        # the cost of one wasted 128x128 exp tile per (pair, head).
        # Heads are processed in GROUPS of 4, software-pipelined: the A@V /
        # normalize phase of group i runs while the score/exp phase of
        # group i+1 occupies the PE, so exp+mask latency never stalls it.
        def emit_phase1(pc, h):
            qb0, qb1, W0, W1, NKB = pc["qb0"], pc["qb1"], pc["W0"], pc["W1"], pc["NKB"]
            fq = h // 2
            po = 64 * (h % 2)
            kTh = kT_sb[fq][po:po + 64, :]
            qsl2 = qT_sb[fq][po:po + 64, qb0 * 128:qb0 * 128 + 256]

            # score rows s[q, k] for both blocks
            E2 = []
            for qb, W in ((qb0, W0), (qb1, W1)):
                ps_s = psum.tile([128, 1024], f32, tag="ps_wide", name="ps_s")
                qsl = qT_sb[fq][po:po + 64, ts(qb, 128)]
                for c0 in range(0, W, 512):
                    cw = min(512, W - c0)
                    nc.tensor.matmul(
                        ps_s[:, c0:c0 + cw], qsl, kTh[:, c0:c0 + cw],
                        start=True, stop=True,
                    )
                E = work.tile([128, S], bf16, tag="E", name="E", bufs=12)
                nc.scalar.activation(E[:, :W], ps_s[:, :W], Exp)
                # zero the masked (k > q) part of the diagonal block
                nc.gpsimd.affine_select(
                    out=E[:, ts(qb, 128)], in_=E[:, ts(qb, 128)],
                    compare_op=mybir.AluOpType.is_ge, fill=0.0,
                    base=0, pattern=[[-1, 128]], channel_multiplier=1,
                )
                E2.append(E)
            pc["Es"][h] = E2

            # transposed scores sT[k, q0|q1] -> exp -> ET (unnormalized)
            # layout: ET[:, kb*256 + 0:128] is (kb, qb0), +128:256 is (kb, qb1)
            ET = work.tile([128, 2048], bf16, tag="ET", name="ET", bufs=6)
            for k0 in range(0, NKB, 4):
                kn = min(4, NKB - k0)
                ps_t = psum.tile([128, 1024], f32, tag="ps_t", name="ps_t")
                for j in range(kn):
                    kb = k0 + j
                    nc.tensor.matmul(
                        ps_t[:, j * 256:(j + 1) * 256],
                        kTh[:, ts(kb, 128)], qsl2,
                        start=True, stop=True,
                    )
                nc.scalar.activation(
                    ET[:, k0 * 256:(k0 + kn) * 256], ps_t[:, :kn * 256], Exp
                )
            # diagonal-block masks: (qb0, qb0) at col 0, (qb1, qb1) at col 128
            nc.gpsimd.affine_select(
                out=ET[:, qb0 * 256:qb0 * 256 + 128],
                in_=ET[:, qb0 * 256:qb0 * 256 + 128],
                compare_op=mybir.AluOpType.is_ge, fill=0.0,
                base=0, pattern=[[1, 128]], channel_multiplier=-1,
            )
            nc.gpsimd.affine_select(
                out=ET[:, qb1 * 256 + 128:qb1 * 256 + 256],
                in_=ET[:, qb1 * 256 + 128:qb1 * 256 + 256],
                compare_op=mybir.AluOpType.is_ge, fill=0.0,
                base=0, pattern=[[1, 128]], channel_multiplier=-1,
            )
            pc["ETs"][h] = ET

        def emit_phase2(pc, h):
            qb0, qb1, W0, W1 = pc["qb0"], pc["qb1"], pc["W0"], pc["W1"]
            ET = pc["ETs"][h]
            for qi, (qb, W, attn_t) in enumerate(
                ((qb0, W0, pc["attn0"]), (qb1, W1, pc["attn1"]))
            ):
                E = pc["Es"][h][qi]
                # a[q, 0:64] = ET.T @ v ; a[q, 64] = row-sum (ones col)
                ps_a = psum.tile([128, VW], f32, tag="ps_wide", name="ps_a")
                for kb in range(qb + 1):
                    nc.tensor.matmul(
                        ps_a[:], ET[:, kb * 256 + qi * 128:kb * 256 + qi * 128 + 128],
                        v_sb[kb][:, VW * h:VW * (h + 1)],
                        start=(kb == 0), stop=(kb == qb),
                    )
                rcp = small.tile([128, 1], f32, tag="rcp", name="rcp")
                nc.vector.reciprocal(rcp[:], ps_a[:, HD:VW])

                wrow = work.tile([128, S], bf16, tag="wrow", name="wrow")
                nc.vector.tensor_scalar_mul(wrow[:, :W], E[:, :W], rcp[:])
                nc.sync.dma_start(w_out[h, ts(qb, 128), 0:W], wrow[:, :W])
                nc.vector.tensor_scalar_mul(
                    attn_t[:, HD * h:HD * (h + 1)], ps_a[:, 0:HD], rcp[:]
                )

        # per pair, per 4-head group: phase1 then phase2; previous pair's
        # transposes after the first phase1, its projection at pair end.
        prev_pc = None
        prev_aTt = None
        for qg in reversed(range(NB // 2)):
            qb0, qb1 = 2 * qg, 2 * qg + 1
            pc = {
                "qb0": qb0, "qb1": qb1,
                "W0": (qb0 + 1) * 128, "W1": (qb1 + 1) * 128,
                "NKB": qb1 + 1,
                "attn0": work.tile([128, GD], bf16, tag="attn", name="attn0", bufs=4),
                "attn1": work.tile([128, GD], bf16, tag="attn", name="attn1", bufs=4),
                "Es": {}, "ETs": {},
            }
            for hg in range(2):
                for h in range(4 * hg, 4 * hg + 4):
                    emit_phase1(pc, h)
                if hg == 0 and prev_pc is None:
                    # first pair: the v-projection chains fill the PE here
                    for tt in range(NB):
                        emit_v(tt)
                if hg == 0 and prev_pc is not None:
                    prev_aTt = [
                        emit_transposes(prev_pc["attn0"]),
                        emit_transposes(prev_pc["attn1"]),
                    ]
                for h in range(4 * hg, 4 * hg + 4):
                    emit_phase2(pc, h)
            if prev_pc is not None:
                emit_proj(prev_aTt[0], prev_pc["qb0"])
                emit_proj(prev_aTt[1], prev_pc["qb1"])
            prev_pc = pc

        emit_proj(emit_transposes(prev_pc["attn0"]), prev_pc["qb0"])
        emit_proj(emit_transposes(prev_pc["attn1"]), prev_pc["qb1"])

    nc.compile()
    return nc


def _numpy_reference(x, mask, w_attn, b_attn, w_proj, b_proj):
    # Defensive fallback for a nonzero padding mask (the problem spec fills
    # it with zeros); replicates the reference math in f32 numpy.
    b, s, d = x.shape
    qkv = x @ w_attn + b_attn
    q, k, v = np.split(qkv, 3, axis=2)

    def split_heads(t):
        return t.reshape(b, s, H, HD).transpose(0, 2, 1, 3)

    q, k, v = split_heads(q), split_heads(k), split_heads(v)
    w = np.einsum("bhqd,bhkd->bhqk", q, k) / np.sqrt(np.float32(HD))
    i = np.arange(s)[:, None]
    j = np.arange(s)[None, :]
    bmask = (i >= j).astype(w.dtype)[None, None]
    w = w * bmask - NEG * (1.0 - bmask)
    w = w + mask
    w = w - w.max(axis=-1, keepdims=True)
    e = np.exp(w)
    w = e / e.sum(axis=-1, keepdims=True)
    a = np.einsum("bhqk,bhkd->bhqd", w, v)
    a = a.transpose(0, 2, 1, 3).reshape(b, s, d)
    a = a @ w_proj + b_proj
    return a.astype(np.float32), w.astype(np.float32)


def kernel(**inputs):
    x = np.asarray(inputs["x"], dtype=np.float32)
    mask = np.asarray(inputs["mask"], dtype=np.float32)
    w_attn = np.asarray(inputs["w_attn"], dtype=np.float32)
    b_attn = np.asarray(inputs["b_attn"], dtype=np.float32)
    w_proj = np.asarray(inputs["w_proj"], dtype=np.float32)
    b_proj = np.asarray(inputs["b_proj"], dtype=np.float32)

    if np.any(mask != 0.0):
        return _numpy_reference(x, mask, w_attn, b_attn, w_proj, b_proj)

    from concourse.bass_utils import run_bass_kernel_spmd

    if "nc" not in _CACHE:
        _CACHE["nc"] = _build_bass()
    nc = _CACHE["nc"]

    bf = ml_dtypes.bfloat16
    in_maps = []
    for c in range(NCORES):
        b, g = divmod(c, 2)
        sl = slice(GD * g, GD * (g + 1))
        in_maps.append({
            "xT": np.ascontiguousarray(x[b].T).astype(bf),
            "qw": np.ascontiguousarray(w_attn[:, sl] * 0.125).astype(bf),
            "kw": np.ascontiguousarray(w_attn[:, D + GD * g:D + GD * (g + 1)]).astype(bf),
            "vw": np.ascontiguousarray(w_attn[:, 2 * D + GD * g:2 * D + GD * (g + 1)]).astype(bf),
            "qbias": np.ascontiguousarray(b_attn[0, sl] * 0.125).reshape(GD, 1).astype(np.float32),
            "kbias": np.ascontiguousarray(b_attn[0, D + GD * g:D + GD * (g + 1)]).reshape(GD, 1).astype(np.float32),
            "wp": np.ascontiguousarray(w_proj[sl, :]).astype(bf),
        })

    res = run_bass_kernel_spmd(nc, in_maps, core_ids=list(range(NCORES))).results

    w = np.empty((B, H, S, S), np.float32)
    a = np.empty((B, S, D), np.float32)
    for c in range(NCORES):
        b, g = divmod(c, 2)
        w[b, G * g:G * (g + 1)] = res[c]["w_out"].astype(np.float32)
    for b in range(B):
        a[b] = res[2 * b]["a_out"] + res[2 * b + 1]["a_out"]
    # exact host-side bias folds: v-bias contributes (sum_k w = 1) b_v @ w_proj
    a += (b_proj[0] + b_attn[0, 2 * D:] @ w_proj).reshape(1, 1, D)
    return a, w
